# revision 5
# baseline (speedup 1.0000x reference)
"""Grouped-query attention with sliding-window mask on 8 Trainium2 cores.

Sharding: core c handles (batch b = c // 4, kv-head group hk = c % 4).
Each core projects q (4 query heads) / k / v for its group, applies RoPE,
runs windowed attention, and computes a partial output projection
out_partial = attn_heads @ Wo[hk block] in fp16.  The host sums the 4
partials per batch in fp32.

v2 (all-bf16, fused pipeline):
- every matmul operand is bf16 (same PE rate as fp32r at these sizes,
  but FWL halves LDWEIGHTS time and DMA traffic halves).
- phase A (projection+RoPE) and phase B (attention+out-proj) are fused
  into one loop over the 16 row tiles, sharing a single PSUM pool set:
  B(T) consumes the q/k/v blocks produced by A(T).
- q/k head blocks are transposed with DMA-engine xbar transposes
  (dma_start_transpose, 2-byte dtype) instead of TensorE transposes,
  removing 80 matmuls + 80 scalar copies from the critical engines.
- Wq/Wk columns are permuted on the host (evens then odds per head) so
  RoPE pairs become contiguous halves; the q.k dot product is invariant
  because both q and k use the same permutation.
- softmax: scores are bounded, so no max subtraction.  exp blocks are
  pre-added on DVE and a single ones-vector matmul yields the
  denominator; the reciprocal is partition-broadcast on gpsimd and
  folded into the PSUM->SBUF move of the attention output.
- projections for q and k/v share the same stationary x chunk
  back-to-back, and the denominator/attention/out-proj matmuls all run
  at moving dim 512.
"""

import sys

for _p in ("/opt/trn_rl_repo", "/opt/pypackages"):
    if _p not in sys.path:
        sys.path.insert(0, _p)

from contextlib import ExitStack

import numpy as np
import ml_dtypes

import concourse.bacc as bacc
import concourse.bass as bass
import concourse.mybir as mybir
import concourse.tile as tile
from concourse.bass_utils import run_bass_kernel_spmd

B, N, E = 2, 2048, 2048
H, G, WIN = 16, 4, 256
HK = H // G          # 4 kv heads
D = E // H           # 128
SCALE = D ** -0.5
NCORES = 8
P = 128
NT = N // P          # 16 row tiles
EC = E // P          # 16 contraction chunks
QTR = N // 4         # 512: x streamed in quarter-columns
HD = D // 2          # 64
F32 = mybir.dt.float32
BF16 = mybir.dt.bfloat16
FP16 = mybir.dt.float16
MASK_VAL = -1.0e30
NPBF16 = ml_dtypes.bfloat16

_compiled = {}


def _rope(nc, rtmp, dst, src_ap, c_ap, s_ap, nb):
    """RoPE on [128, nb*128] where each 128-block is [x0(64) | x1(64)]
    (host permuted the projection weights to evens-then-odds order).

    dst[., b, 0:64]   = x0*cos - x1*sin
    dst[., b, 64:128] = x0*sin + x1*cos
    """
    sv = src_ap.rearrange("p (b d) -> p b d", d=D)
    dv = dst[:].rearrange("p (b d) -> p b d", d=D)
    x0 = sv[:, :, 0:HD]
    x1 = sv[:, :, HD:D]
    d0 = dv[:, :, 0:HD]
    d1 = dv[:, :, HD:D]
    c3 = bass.AP(c_ap.tensor, c_ap.offset, [c_ap.ap[0], [0, nb], c_ap.ap[1]])
    s3 = bass.AP(s_ap.tensor, s_ap.offset, [s_ap.ap[0], [0, nb], s_ap.ap[1]])
    t0 = rtmp.tile([P, nb * HD], F32, tag="t0", name="t0")
    t1 = rtmp.tile([P, nb * HD], F32, tag="t1", name="t1")
    t0v = t0[:].rearrange("p (b d) -> p b d", d=HD)
    t1v = t1[:].rearrange("p (b d) -> p b d", d=HD)
    nc.vector.tensor_mul(t0v, x0, c3)
    nc.vector.tensor_mul(t1v, x1, s3)
    nc.vector.tensor_sub(d0, t0v, t1v)
    t2 = rtmp.tile([P, nb * HD], F32, tag="t2", name="t2")
    t3 = rtmp.tile([P, nb * HD], F32, tag="t3", name="t3")
    t2v = t2[:].rearrange("p (b d) -> p b d", d=HD)
    t3v = t3[:].rearrange("p (b d) -> p b d", d=HD)
    nc.vector.tensor_mul(t2v, x0, s3)
    nc.vector.tensor_mul(t3v, x1, c3)
    nc.vector.tensor_add(d1, t2v, t3v)


def _bcast_g(ap):
    """[128, 128] AP -> [128, G, 128] with a 0-step head dim."""
    return bass.AP(ap.tensor, ap.offset, [ap.ap[0], [0, G], ap.ap[1]])


def _build():
    nc = bacc.Bacc("TRN2", target_bir_lowering=False, debug=False)

    xt_d = nc.dram_tensor("xt", [E, N], BF16, kind="ExternalInput")
    wq_d = nc.dram_tensor("wq", [E, G * D], BF16, kind="ExternalInput")
    wkv_d = nc.dram_tensor("wkv", [E, 2 * D], BF16, kind="ExternalInput")
    wo_d = nc.dram_tensor("wo", [G * D, E], BF16, kind="ExternalInput")
    cos_d = nc.dram_tensor("coss", [N, HD], F32, kind="ExternalInput")
    sin_d = nc.dram_tensor("sins", [N, HD], F32, kind="ExternalInput")
    maskt_d = nc.dram_tensor("maskt", [P, 2 * P], F32, kind="ExternalInput")
    onesr_d = nc.dram_tensor("onesr", [P, 1], BF16, kind="ExternalInput")
    out_d = nc.dram_tensor("out", [N, E], FP16, kind="ExternalOutput")

    # DRAM views: chunked by 128-row groups with partition innermost
    xt3 = xt_d.ap().rearrange("(c p) n -> p c n", p=P)       # [128, 16, 2048]
    wq3 = wq_d.ap().rearrange("(c p) m -> p c m", p=P)       # [128, 16, 512]
    wkv3 = wkv_d.ap().rearrange("(c p) m -> p c m", p=P)     # [128, 16, 256]
    wo3 = wo_d.ap().rearrange("(g p) e -> p g e", p=P)       # [128, 4, 2048]

    W = G * P  # 512: (g, qi) moving width

    with tile.TileContext(nc) as tc, ExitStack() as top:
        pers = top.enter_context(tc.tile_pool(name="pers", bufs=1))
        wq_sb = pers.tile([P, EC * W], BF16, tag="wq")        # [p, (e, 512)]
        wkv_sb = pers.tile([P, EC * 2 * D], BF16, tag="wkv")  # [p, (e, 256)]
        wo_sb = pers.tile([P, G * E], BF16, tag="wo")         # [p, (g, 2048)]
        cos_sb = pers.tile([P, NT * HD], F32, tag="cos")
        sin_sb = pers.tile([P, NT * HD], F32, tag="sin")
        maskt_sb = pers.tile([P, 2 * P], F32, tag="maskt")
        ones_sb = pers.tile([P, 1], BF16, tag="ones")
        kt_sb = pers.tile([P, N], BF16, tag="kt")             # [d, n]
        v_sb = pers.tile([P, N], BF16, tag="v")               # blk t: v[t*128+p, d]

        wq_v = wq_sb[:].rearrange("p (c m) -> p c m", m=W)
        wkv_v = wkv_sb[:].rearrange("p (c m) -> p c m", m=2 * D)
        wo_v = wo_sb[:].rearrange("p (g e) -> p g e", e=E)

        xt_pool = top.enter_context(tc.tile_pool(name="xtp", bufs=32))
        qt_pool = top.enter_context(tc.tile_pool(name="qtp", bufs=3))
        qrot_pool = top.enter_context(tc.tile_pool(name="qrot", bufs=3))
        krot_pool = top.enter_context(tc.tile_pool(name="krot", bufs=3))
        rtmp = top.enter_context(tc.tile_pool(name="rtmp", bufs=4))
        ex_pool = top.enter_context(tc.tile_pool(name="ex", bufs=6))
        es_pool = top.enter_context(tc.tile_pool(name="es", bufs=4))
        smm_pool = top.enter_context(tc.tile_pool(name="smm", bufs=2))
        stat_pool = top.enter_context(tc.tile_pool(name="stat", bufs=2))
        bc_pool = top.enter_context(tc.tile_pool(name="bcs", bufs=2))
        ao_pool = top.enter_context(tc.tile_pool(name="aosb", bufs=2))
        osb_pool = top.enter_context(tc.tile_pool(name="osb", bufs=2))

        ps_big = top.enter_context(tc.tile_pool(name="psb", bufs=5, space="PSUM"))
        ps_kv = top.enter_context(tc.tile_pool(name="psk", bufs=2, space="PSUM"))
        ps_den = top.enter_context(tc.tile_pool(name="psd", bufs=1, space="PSUM"))

        xts = [None] * NT  # per-chunk x tiles, index [T//4 quarter][e] flat
        qt_tiles = [None] * NT

        def a_part(T):
            qtr, tq = divmod(T, 4)
            # ---- DMA issue (x quarters + first-tile interleaved weights)
            if tq == 0:
                xts_cur = [xt_pool.tile([P, QTR], BF16, tag="xt", name="xtt")
                           for _ in range(EC)]
                xts[qtr] = xts_cur
                if T == 0:
                    # order: everything A(0)/B(0) needs, roughly in PE
                    # consumption order; DMA queues run in parallel anyway
                    nc.sync.dma_start(wq_v[:, 0:4, :], wq3[:, 0:4, :])
                    nc.sync.dma_start(wkv_v[:, 0:8, :], wkv3[:, 0:8, :])
                    for e in range(4):
                        nc.sync.dma_start(
                            xts_cur[e][:], xt3[:, e, 0:QTR])
                    nc.sync.dma_start(
                        cos_sb[:].rearrange("p (t d) -> p t d", d=HD),
                        cos_d.ap().rearrange("(t p) d -> p t d", p=P))
                    nc.sync.dma_start(
                        sin_sb[:].rearrange("p (t d) -> p t d", d=HD),
                        sin_d.ap().rearrange("(t p) d -> p t d", p=P))
                    nc.sync.dma_start(wq_v[:, 4:8, :], wq3[:, 4:8, :])
                    nc.sync.dma_start(wkv_v[:, 8:16, :], wkv3[:, 8:16, :])
                    for e in range(4, 8):
                        nc.sync.dma_start(
                            xts_cur[e][:], xt3[:, e, 0:QTR])
                    nc.sync.dma_start(maskt_sb[:], maskt_d.ap())
                    nc.sync.dma_start(ones_sb[:], onesr_d.ap())
                    nc.sync.dma_start(wq_v[:, 8:12, :], wq3[:, 8:12, :])
                    for e in range(8, 12):
                        nc.sync.dma_start(
                            xts_cur[e][:], xt3[:, e, 0:QTR])
                    nc.sync.dma_start(wq_v[:, 12:16, :], wq3[:, 12:16, :])
                    for e in range(12, 16):
                        nc.sync.dma_start(
                            xts_cur[e][:], xt3[:, e, 0:QTR])
                    nc.sync.dma_start(wo_v[:, 0:2, :], wo3[:, 0:2, :])
                    nc.sync.dma_start(wo_v[:, 2:4, :], wo3[:, 2:4, :])
                else:
                    for e in range(EC):
                        nc.sync.dma_start(
                            xts_cur[e][:],
                            xt3[:, e, qtr * QTR:(qtr + 1) * QTR])
            xts_cur = xts[qtr]

            # ---- A: projections (shared stationary x chunk) + RoPE
            q_ps = ps_big.tile([P, W], F32, tag="psb")
            kv_ps = ps_kv.tile([P, 2 * D], F32, tag="psk")
            for e in range(EC):
                lhsT = xts_cur[e][:, tq * P:(tq + 1) * P]
                nc.tensor.matmul(
                    q_ps[:], lhsT, wq_v[:, e, :],
                    start=(e == 0), stop=(e == EC - 1))
                nc.tensor.matmul(
                    kv_ps[:], lhsT, wkv_v[:, e, :],
                    start=(e == 0), stop=(e == EC - 1))

            c_ap = cos_sb[:, T * HD:(T + 1) * HD]
            s_ap = sin_sb[:, T * HD:(T + 1) * HD]
            q_rot = qrot_pool.tile([P, W], BF16, tag="qrot")
            k_rot = krot_pool.tile([P, D], BF16, tag="krot")
            _rope(nc, rtmp, q_rot, q_ps[:], c_ap, s_ap, G)
            _rope(nc, rtmp, k_rot, kv_ps[:, 0:D], c_ap, s_ap, 1)
            nc.vector.tensor_copy(v_sb[:, T * P:(T + 1) * P], kv_ps[:, D:2 * D])

            # ---- transposes on the DMA xbar (2-byte dtype)
            qt_t = qt_pool.tile([P, W], BF16, tag="qt")
            qt_tiles[T] = qt_t
            nc.sync.dma_start_transpose(
                qt_t[:].rearrange("p (g n) -> p g n", g=G), q_rot[:])
            nc.sync.dma_start_transpose(kt_sb[:, T * P:(T + 1) * P], k_rot[:])

        def b_part(qt):
            # ---- B: attention for query tile qt
            qt_t = qt_tiles[qt]
            nk = min(qt, 2) + 1
            kb0 = qt - (nk - 1)
            exps = [ex_pool.tile([P, W], BF16, tag="ex", name="exv")
                    for _ in range(nk)]
            for j in range(nk):
                kb = kb0 + j
                dabs = kb - qt          # -2, -1, or 0
                st_ps = ps_big.tile([P, W], F32, tag="psb")
                nc.tensor.matmul(
                    st_ps[:], kt_sb[:, kb * P:(kb + 1) * P], qt_t[:],
                    start=True, stop=True)
                if dabs == -1:
                    nc.scalar.activation(
                        exps[j][:], st_ps[:], mybir.ActivationFunctionType.Exp)
                else:
                    mblk = maskt_sb[:, 0:P] if dabs == -2 \
                        else maskt_sb[:, P:2 * P]
                    st_sb = smm_pool.tile([P, W], F32, tag="stsb")
                    nc.vector.tensor_add(
                        st_sb[:].rearrange("p (g q) -> p g q", g=G),
                        st_ps[:].rearrange("p (g q) -> p g q", g=G),
                        _bcast_g(mblk))
                    nc.scalar.activation(
                        exps[j][:], st_sb[:], mybir.ActivationFunctionType.Exp)

            # denominator: DVE pre-add, single ones matmul, reciprocal
            if nk == 1:
                esum_ap = exps[0][:]
            elif nk == 2:
                es = es_pool.tile([P, W], BF16, tag="es", name="es")
                nc.vector.tensor_add(es[:], exps[0][:], exps[1][:])
                esum_ap = es[:]
            else:
                e01 = es_pool.tile([P, W], BF16, tag="es", name="e01")
                nc.vector.tensor_add(e01[:], exps[0][:], exps[1][:])
                es = es_pool.tile([P, W], BF16, tag="es", name="e012")
                nc.vector.tensor_add(es[:], e01[:], exps[2][:])
                esum_ap = es[:]
            den_ps = ps_den.tile([1, W], F32, tag="psd")
            nc.tensor.matmul(den_ps[:], ones_sb[:, 0:1], esum_ap,
                             start=True, stop=True)
            recip = stat_pool.tile([1, W], F32, tag="recip")
            nc.vector.reciprocal_approx_fast(recip[:], den_ps[:])
            bc_sb = bc_pool.tile([P, W], F32, tag="bcsb")
            nc.gpsimd.partition_broadcast(bc_sb[:], recip[:])

            ao_ps = ps_big.tile([P, W], F32, tag="psb")
            for j in range(nk):
                kb = kb0 + j
                nc.tensor.matmul(
                    ao_ps[:], v_sb[:, kb * P:(kb + 1) * P], exps[j][:],
                    start=(j == 0), stop=(j == nk - 1))
            ao_sb = ao_pool.tile([P, W], BF16, tag="aosb")
            nc.vector.tensor_mul(ao_sb[:], ao_ps[:], bc_sb[:])

            out_sb = osb_pool.tile([P, E], FP16, tag="outsb")
            for eb in range(4):
                wo_ps = ps_big.tile([P, W], F32, tag="psb")
                for g in range(G):
                    nc.tensor.matmul(
                        wo_ps[:],
                        ao_sb[:, g * P:(g + 1) * P],
                        wo_v[:, g, eb * W:(eb + 1) * W],
                        start=(g == 0), stop=(g == G - 1))
                if eb % 2 == 0:
                    nc.scalar.copy(out_sb[:, eb * W:(eb + 1) * W], wo_ps[:])
                else:
                    nc.vector.tensor_copy(out_sb[:, eb * W:(eb + 1) * W],
                                          wo_ps[:])
            nc.scalar.dma_start(out_d.ap()[qt * P:(qt + 1) * P, :], out_sb[:])

        # software-pipelined: B runs one tile behind A so the in-order PE
        # queue has A(T+1) work buffered while B(T) waits on wo at startup
        for step in range(NT + 1):
            if step < NT:
                a_part(step)
            if step >= 1:
                b_part(step - 1)

    nc.compile()
    return nc


_PERM = np.concatenate([np.arange(0, D, 2), np.arange(1, D, 2)])


def _host_inputs(x, rope_cos, rope_sin, Wq, Wk, Wv, Wo):
    """Build the 8 per-core input maps (bf16 weights, permuted RoPE lanes)."""
    band = np.full((P, 3 * P), MASK_VAL, dtype=np.float32)
    r = np.arange(P)[:, None]
    c = np.arange(3 * P)[None, :]
    band[(c > r) & (c <= r + WIN)] = 0.0
    # transposed mask blocks: [:, :128] for key-tile offset -2,
    # [:, 128:] (causal) for offset 0
    maskt = np.ascontiguousarray(np.concatenate(
        [band[:, 0:P].T, band[:, 2 * P:3 * P].T], axis=1))

    def permute_heads(w):
        # w: [E, nh*D] -> permute each head's columns to evens-then-odds
        nh = w.shape[1] // D
        w = w.reshape(E, nh, D)
        return w[:, :, _PERM].reshape(E, nh * D)

    in_maps = []
    for core in range(NCORES):
        b, hk = divmod(core, HK)
        xt = np.ascontiguousarray(x[b].T).astype(NPBF16)
        wq = np.ascontiguousarray(permute_heads(
            Wq[:, hk * G * D:(hk + 1) * G * D] * SCALE)).astype(NPBF16)
        wk = permute_heads(Wk[:, hk * D:(hk + 1) * D])
        wkv = np.ascontiguousarray(np.concatenate(
            [wk, Wv[:, hk * D:(hk + 1) * D]], axis=1)).astype(NPBF16)
        wo = np.ascontiguousarray(
            Wo[hk * G * D:(hk + 1) * G * D, :]).astype(NPBF16)
        in_maps.append({
            "xt": xt,
            "wq": wq,
            "wkv": wkv,
            "wo": wo,
            "coss": np.ascontiguousarray(rope_cos[b].astype(np.float32)),
            "sins": np.ascontiguousarray(rope_sin[b].astype(np.float32)),
            "maskt": maskt,
            "onesr": np.ones((P, 1), dtype=NPBF16),
        })
    return in_maps


def _run(inputs, trace=False, **kw):
    if "nc" not in _compiled:
        _compiled["nc"] = _build()
    nc = _compiled["nc"]
    in_maps = _host_inputs(**inputs)
    res = run_bass_kernel_spmd(nc, in_maps, list(range(NCORES)), trace=trace, **kw)
    out = np.zeros((B, N, E), dtype=np.float32)
    for core in range(NCORES):
        b = core // HK
        out[b] += np.asarray(res.results[core]["out"]).astype(np.float32)
    return out, res


def kernel(**inputs):
    out, _ = _run(inputs, trace=False)
    return out


# revision 6
# speedup vs baseline: 1.2699x; 1.2699x over previous
"""Grouped-query attention with sliding-window mask on 8 Trainium2 cores.

Sharding: core c handles (batch b = c // 4, kv-head group hk = c % 4).
Each core projects q (4 query heads) / k / v for its group, applies RoPE,
runs windowed attention, and computes a partial output projection
out_partial = attn_heads @ Wo[hk block] in fp16.  The host sums the 4
partials per batch in fp32.

v3 (all-bf16, 4-slot interleaved pipeline):
- every matmul operand is bf16 (same PE rate as fp32r at these sizes, but
  FWL halves LDWEIGHTS and DMA traffic halves).
- per step s the PE stream is: scores(s-1) | projections(s) | den+attnV
  (s-1) | out-proj(s-2).  Each matmul group's cross-engine inputs were
  produced >= half a step earlier, so the PE never waits on the
  softmax/RoPE chain and HAM stays warm.
- all DRAM inputs are pre-laid out host-side as the exact SBUF tile
  images (16 KB contiguous per partition row), so a whole tensor loads
  as one large DMA that spreads across all 16 SDMA engines.
- q/k head blocks are transposed with DMA xbar transposes
  (dma_start_transpose, 2-byte dtype) on the ACT ring; bulk loads ride
  the SP ring so the latency-critical transposes never queue behind
  them.
- Wq/Wk columns are permuted on the host (evens then odds per head) so
  RoPE pairs become contiguous halves; q.k dot products are invariant.
- softmax: scores are bounded, so no max subtraction.  exp blocks are
  pre-added on DVE and one ones-vector matmul yields the denominator;
  the reciprocal is partition-broadcast on gpsimd and folded into the
  PSUM->SBUF move of the attention output.
"""

import sys

for _p in ("/opt/trn_rl_repo", "/opt/pypackages"):
    if _p not in sys.path:
        sys.path.insert(0, _p)

from contextlib import ExitStack

import numpy as np
import ml_dtypes

import concourse.bacc as bacc
import concourse.bass as bass
import concourse.mybir as mybir
import concourse.tile as tile
from concourse.bass_utils import run_bass_kernel_spmd

B, N, E = 2, 2048, 2048
H, G, WIN = 16, 4, 256
HK = H // G          # 4 kv heads
D = E // H           # 128
SCALE = D ** -0.5
NCORES = 8
P = 128
NT = N // P          # 16 row tiles
EC = E // P          # 16 contraction chunks
QTR = N // 4         # 512: x streamed in quarter-columns
HD = D // 2          # 64
W = G * P            # 512: (g, qi) moving width
F32 = mybir.dt.float32
BF16 = mybir.dt.bfloat16
FP16 = mybir.dt.float16
MASK_VAL = -1.0e30
NPBF16 = ml_dtypes.bfloat16

_compiled = {}


def _rope(nc, rtmp, dst, src_ap, c_ap, s_ap, nb):
    """RoPE on [128, nb*128] where each 128-block is [x0(64) | x1(64)]
    (host permuted the projection weights to evens-then-odds order)."""
    sv = src_ap.rearrange("p (b d) -> p b d", d=D)
    dv = dst[:].rearrange("p (b d) -> p b d", d=D)
    x0 = sv[:, :, 0:HD]
    x1 = sv[:, :, HD:D]
    d0 = dv[:, :, 0:HD]
    d1 = dv[:, :, HD:D]
    c3 = bass.AP(c_ap.tensor, c_ap.offset, [c_ap.ap[0], [0, nb], c_ap.ap[1]])
    s3 = bass.AP(s_ap.tensor, s_ap.offset, [s_ap.ap[0], [0, nb], s_ap.ap[1]])
    t0 = rtmp.tile([P, nb * HD], F32, tag="t0", name="t0")
    t1 = rtmp.tile([P, nb * HD], F32, tag="t1", name="t1")
    t0v = t0[:].rearrange("p (b d) -> p b d", d=HD)
    t1v = t1[:].rearrange("p (b d) -> p b d", d=HD)
    nc.vector.tensor_mul(t0v, x0, c3)
    nc.vector.tensor_mul(t1v, x1, s3)
    nc.vector.tensor_sub(d0, t0v, t1v)
    t2 = rtmp.tile([P, nb * HD], F32, tag="t2", name="t2")
    t3 = rtmp.tile([P, nb * HD], F32, tag="t3", name="t3")
    t2v = t2[:].rearrange("p (b d) -> p b d", d=HD)
    t3v = t3[:].rearrange("p (b d) -> p b d", d=HD)
    nc.vector.tensor_mul(t2v, x0, s3)
    nc.vector.tensor_mul(t3v, x1, c3)
    nc.vector.tensor_add(d1, t2v, t3v)


def _bcast_g(ap):
    """[128, 128] AP -> [128, G, 128] with a 0-step head dim."""
    return bass.AP(ap.tensor, ap.offset, [ap.ap[0], [0, G], ap.ap[1]])


def _build():
    nc = bacc.Bacc("TRN2", target_bir_lowering=False, debug=False)

    # all inputs are SBUF tile images: [128 partitions, free]
    xt_d = nc.dram_tensor("xti", [4, P, EC * QTR], BF16, kind="ExternalInput")
    wq_d = nc.dram_tensor("wqi", [P, EC * W], BF16, kind="ExternalInput")
    wkv_d = nc.dram_tensor("wkvi", [P, EC * 2 * D], BF16, kind="ExternalInput")
    wo_d = nc.dram_tensor("woi", [P, G * E], BF16, kind="ExternalInput")
    cos_d = nc.dram_tensor("cosi", [P, NT * HD], F32, kind="ExternalInput")
    sin_d = nc.dram_tensor("sini", [P, NT * HD], F32, kind="ExternalInput")
    maskt_d = nc.dram_tensor("maskt", [P, 2 * P], F32, kind="ExternalInput")
    onesr_d = nc.dram_tensor("onesr", [P, 1], BF16, kind="ExternalInput")
    out_d = nc.dram_tensor("out", [N, E], FP16, kind="ExternalOutput")

    with tile.TileContext(nc) as tc, ExitStack() as top:
        pers = top.enter_context(tc.tile_pool(name="pers", bufs=1))
        wq_sb = pers.tile([P, EC * W], BF16, tag="wq")        # [p, (e, 512)]
        wkv_sb = pers.tile([P, EC * 2 * D], BF16, tag="wkv")  # [p, (e, 256)]
        wo_sb = pers.tile([P, G * E], BF16, tag="wo")         # [p, (g, 2048)]
        cos_sb = pers.tile([P, NT * HD], F32, tag="cos")
        sin_sb = pers.tile([P, NT * HD], F32, tag="sin")
        maskt_sb = pers.tile([P, 2 * P], F32, tag="maskt")
        ones_sb = pers.tile([P, 1], BF16, tag="ones")
        kt_sb = pers.tile([P, N], BF16, tag="kt")             # [d, n]
        v_sb = pers.tile([P, N], BF16, tag="v")               # blk t: v[t*128+p, d]

        wq_v = wq_sb[:].rearrange("p (c m) -> p c m", m=W)
        wkv_v = wkv_sb[:].rearrange("p (c m) -> p c m", m=2 * D)
        wo_v = wo_sb[:].rearrange("p (g e) -> p g e", e=E)

        xq_pool = top.enter_context(tc.tile_pool(name="xqp", bufs=2))
        qt_pool = top.enter_context(tc.tile_pool(name="qtp", bufs=3))
        qrot_pool = top.enter_context(tc.tile_pool(name="qrot", bufs=3))
        krot_pool = top.enter_context(tc.tile_pool(name="krot", bufs=3))
        rtmp = top.enter_context(tc.tile_pool(name="rtmp", bufs=4))
        ex_pool = top.enter_context(tc.tile_pool(name="ex", bufs=6))
        es_pool = top.enter_context(tc.tile_pool(name="es", bufs=4))
        smm_pool = top.enter_context(tc.tile_pool(name="smm", bufs=2))
        stat_pool = top.enter_context(tc.tile_pool(name="stat", bufs=2))
        bc_pool = top.enter_context(tc.tile_pool(name="bcs", bufs=2))
        ao_pool = top.enter_context(tc.tile_pool(name="aosb", bufs=3))
        osb_pool = top.enter_context(tc.tile_pool(name="osb", bufs=2))

        ps_big = top.enter_context(tc.tile_pool(name="psb", bufs=5, space="PSUM"))
        ps_kv = top.enter_context(tc.tile_pool(name="psk", bufs=2, space="PSUM"))
        ps_den = top.enter_context(tc.tile_pool(name="psd", bufs=1, space="PSUM"))

        xqs = [None] * 4        # per-quarter x tiles [128, EC*512]
        qt_tiles = [None] * NT
        st_live = [None] * NT   # per-qt state handed between b_* slots

        def issue_xq(qtr, split):
            xq = xq_pool.tile([P, EC * QTR], BF16, tag="xq", name="xqt")
            xqs[qtr] = xq
            src = xt_d.ap()[qtr]
            if split:
                nc.sync.dma_start(xq[:, 0:2 * QTR], src[:, 0:2 * QTR])
                nc.sync.dma_start(xq[:, 2 * QTR:8 * QTR], src[:, 2 * QTR:8 * QTR])
                nc.sync.dma_start(xq[:, 8 * QTR:16 * QTR], src[:, 8 * QTR:16 * QTR])
            else:
                nc.sync.dma_start(xq[:], src)

        def a_part(T):
            qtr, tq = divmod(T, 4)
            if T == 0:
                # startup: smallest-first so tile 0's deps land early
                nc.sync.dma_start(wq_sb[:, 0:2 * W], wq_d.ap()[:, 0:2 * W])
                nc.sync.dma_start(wkv_sb[:, 0:2 * 2 * D],
                                  wkv_d.ap()[:, 0:2 * 2 * D])
                issue_xq(0, split=True)
                nc.sync.dma_start(cos_sb[:], cos_d.ap())
                nc.sync.dma_start(sin_sb[:], sin_d.ap())
                nc.sync.dma_start(wq_sb[:, 2 * W:8 * W],
                                  wq_d.ap()[:, 2 * W:8 * W])
                nc.sync.dma_start(wkv_sb[:, 2 * 2 * D:16 * 2 * D],
                                  wkv_d.ap()[:, 2 * 2 * D:16 * 2 * D])
                nc.sync.dma_start(maskt_sb[:], maskt_d.ap())
                nc.sync.dma_start(ones_sb[:], onesr_d.ap())
                nc.sync.dma_start(wq_sb[:, 8 * W:16 * W],
                                  wq_d.ap()[:, 8 * W:16 * W])
                nc.sync.dma_start(wo_sb[:, 0:2 * E], wo_d.ap()[:, 0:2 * E])
                nc.sync.dma_start(wo_sb[:, 2 * E:4 * E],
                                  wo_d.ap()[:, 2 * E:4 * E])
            if T in (2, 6, 10):
                issue_xq(T // 4 + 1, split=False)
            xq = xqs[qtr]

            # projections: q and k/v share each stationary x chunk
            q_ps = ps_big.tile([P, W], F32, tag="psb")
            kv_ps = ps_kv.tile([P, 2 * D], F32, tag="psk")
            for e in range(EC):
                lhsT = xq[:, e * QTR + tq * P: e * QTR + (tq + 1) * P]
                nc.tensor.matmul(
                    q_ps[:], lhsT, wq_v[:, e, :],
                    start=(e == 0), stop=(e == EC - 1))
                nc.tensor.matmul(
                    kv_ps[:], lhsT, wkv_v[:, e, :],
                    start=(e == 0), stop=(e == EC - 1))

            c_ap = cos_sb[:, T * HD:(T + 1) * HD]
            s_ap = sin_sb[:, T * HD:(T + 1) * HD]
            q_rot = qrot_pool.tile([P, W], BF16, tag="qrot")
            k_rot = krot_pool.tile([P, D], BF16, tag="krot")
            _rope(nc, rtmp, q_rot, q_ps[:], c_ap, s_ap, G)
            _rope(nc, rtmp, k_rot, kv_ps[:, 0:D], c_ap, s_ap, 1)
            nc.vector.tensor_copy(v_sb[:, T * P:(T + 1) * P], kv_ps[:, D:2 * D])

            # transposes on the DMA xbar; ACT ring (sync ring carries bulk)
            qt_t = qt_pool.tile([P, W], BF16, tag="qt")
            qt_tiles[T] = qt_t
            nc.scalar.dma_start_transpose(
                qt_t[:].rearrange("p (g n) -> p g n", g=G), q_rot[:])
            nc.scalar.dma_start_transpose(kt_sb[:, T * P:(T + 1) * P], k_rot[:])

        def b_scores(qt):
            """scores + exp for query tile qt (inputs ready a step ago)."""
            qt_t = qt_tiles[qt]
            nk = min(qt, 2) + 1
            kb0 = qt - (nk - 1)
            exps = [ex_pool.tile([P, W], BF16, tag="ex", name="exv")
                    for _ in range(nk)]
            for j in range(nk):
                kb = kb0 + j
                dabs = kb - qt          # -2, -1, or 0
                st_ps = ps_big.tile([P, W], F32, tag="psb")
                nc.tensor.matmul(
                    st_ps[:], kt_sb[:, kb * P:(kb + 1) * P], qt_t[:],
                    start=True, stop=True)
                if dabs == -1:
                    nc.scalar.activation(
                        exps[j][:], st_ps[:], mybir.ActivationFunctionType.Exp)
                else:
                    mblk = maskt_sb[:, 0:P] if dabs == -2 \
                        else maskt_sb[:, P:2 * P]
                    st_sb = smm_pool.tile([P, W], F32, tag="stsb")
                    nc.vector.tensor_add(
                        st_sb[:].rearrange("p (g q) -> p g q", g=G),
                        st_ps[:].rearrange("p (g q) -> p g q", g=G),
                        _bcast_g(mblk))
                    nc.scalar.activation(
                        exps[j][:], st_sb[:], mybir.ActivationFunctionType.Exp)
            # denominator pre-add on DVE
            if nk == 1:
                esum_ap = exps[0][:]
            elif nk == 2:
                es = es_pool.tile([P, W], BF16, tag="es", name="es")
                nc.vector.tensor_add(es[:], exps[0][:], exps[1][:])
                esum_ap = es[:]
            else:
                e01 = es_pool.tile([P, W], BF16, tag="es", name="e01")
                nc.vector.tensor_add(e01[:], exps[0][:], exps[1][:])
                es = es_pool.tile([P, W], BF16, tag="es", name="e012")
                nc.vector.tensor_add(es[:], e01[:], exps[2][:])
                esum_ap = es[:]
            st_live[qt] = (exps, esum_ap, nk, kb0)

        def b_tail(qt):
            """denominator matmul, reciprocal, attn@V, normalize."""
            exps, esum_ap, nk, kb0 = st_live[qt]
            den_ps = ps_den.tile([1, W], F32, tag="psd")
            nc.tensor.matmul(den_ps[:], ones_sb[:, 0:1], esum_ap,
                             start=True, stop=True)
            recip = stat_pool.tile([1, W], F32, tag="recip")
            nc.vector.reciprocal_approx_fast(recip[:], den_ps[:])
            bc_sb = bc_pool.tile([P, W], F32, tag="bcsb")
            nc.gpsimd.partition_broadcast(bc_sb[:], recip[:])

            ao_ps = ps_big.tile([P, W], F32, tag="psb")
            for j in range(nk):
                kb = kb0 + j
                nc.tensor.matmul(
                    ao_ps[:], v_sb[:, kb * P:(kb + 1) * P], exps[j][:],
                    start=(j == 0), stop=(j == nk - 1))
            ao_sb = ao_pool.tile([P, W], BF16, tag="aosb")
            nc.vector.tensor_mul(ao_sb[:], ao_ps[:], bc_sb[:])
            st_live[qt] = ao_sb

        def b_wo(qt):
            """output projection + store for query tile qt."""
            ao_sb = st_live[qt]
            out_sb = osb_pool.tile([P, E], FP16, tag="outsb")
            for eb in range(4):
                wo_ps = ps_big.tile([P, W], F32, tag="psb")
                for g in range(G):
                    nc.tensor.matmul(
                        wo_ps[:],
                        ao_sb[:, g * P:(g + 1) * P],
                        wo_v[:, g, eb * W:(eb + 1) * W],
                        start=(g == 0), stop=(g == G - 1))
                if eb % 2 == 0:
                    nc.scalar.copy(out_sb[:, eb * W:(eb + 1) * W], wo_ps[:])
                else:
                    nc.vector.tensor_copy(out_sb[:, eb * W:(eb + 1) * W],
                                          wo_ps[:])
            nc.scalar.dma_start(out_d.ap()[qt * P:(qt + 1) * P, :], out_sb[:])

        # 4-slot pipeline: scores lag A by 1 step, out-proj by 2
        for step in range(NT + 2):
            if 1 <= step <= NT:
                b_scores(step - 1)
            if step < NT:
                a_part(step)
            if 1 <= step <= NT:
                b_tail(step - 1)
            if step >= 2:
                b_wo(step - 2)

    nc.compile()
    return nc


_PERM = np.concatenate([np.arange(0, D, 2), np.arange(1, D, 2)])


def _sbuf_image(w, chunk):
    """[E, M] weight -> [128, (E/128, M)] SBUF image, contiguous rows."""
    rows = w.shape[0]
    return np.ascontiguousarray(
        w.reshape(rows // P, P, chunk).transpose(1, 0, 2).reshape(P, -1))


def _host_inputs(x, rope_cos, rope_sin, Wq, Wk, Wv, Wo):
    """Build the 8 per-core input maps (bf16, SBUF-image layouts)."""
    band = np.full((P, 3 * P), MASK_VAL, dtype=np.float32)
    r = np.arange(P)[:, None]
    c = np.arange(3 * P)[None, :]
    band[(c > r) & (c <= r + WIN)] = 0.0
    maskt = np.ascontiguousarray(np.concatenate(
        [band[:, 0:P].T, band[:, 2 * P:3 * P].T], axis=1))

    def permute_heads(w):
        nh = w.shape[1] // D
        w = w.reshape(E, nh, D)
        return w[:, :, _PERM].reshape(E, nh * D)

    in_maps = []
    for core in range(NCORES):
        b, hk = divmod(core, HK)
        xt = np.ascontiguousarray(x[b].T).astype(NPBF16)      # [E, N]
        # x image: [4 quarters, 128, (e, 512)]
        xti = np.ascontiguousarray(
            xt.reshape(EC, P, 4, QTR).transpose(2, 1, 0, 3).reshape(
                4, P, EC * QTR))
        wq = permute_heads(
            Wq[:, hk * G * D:(hk + 1) * G * D] * SCALE).astype(NPBF16)
        wk = permute_heads(Wk[:, hk * D:(hk + 1) * D])
        wkv = np.concatenate(
            [wk, Wv[:, hk * D:(hk + 1) * D]], axis=1).astype(NPBF16)
        wo = Wo[hk * G * D:(hk + 1) * G * D, :].astype(NPBF16)
        cos = rope_cos[b].astype(np.float32)                  # [N, 64]
        sin = rope_sin[b].astype(np.float32)
        in_maps.append({
            "xti": xti,
            "wqi": _sbuf_image(wq, G * D),
            "wkvi": _sbuf_image(wkv, 2 * D),
            "woi": _sbuf_image(wo, E),
            "cosi": _sbuf_image(cos, HD),
            "sini": _sbuf_image(sin, HD),
            "maskt": maskt,
            "onesr": np.ones((P, 1), dtype=NPBF16),
        })
    return in_maps


def _run(inputs, trace=False, **kw):
    if "nc" not in _compiled:
        _compiled["nc"] = _build()
    nc = _compiled["nc"]
    in_maps = _host_inputs(**inputs)
    res = run_bass_kernel_spmd(nc, in_maps, list(range(NCORES)), trace=trace, **kw)
    out = np.zeros((B, N, E), dtype=np.float32)
    for core in range(NCORES):
        b = core // HK
        out[b] += np.asarray(res.results[core]["out"]).astype(np.float32)
    return out, res


def kernel(**inputs):
    out, _ = _run(inputs, trace=False)
    return out


# revision 10
# speedup vs baseline: 1.3476x; 1.0612x over previous
"""Grouped-query attention with sliding-window mask on 8 Trainium2 cores.

Sharding: core c handles (batch b = c // 4, kv-head group hk = c % 4).
Each core projects q (4 query heads) / k / v for its group, applies RoPE,
runs windowed attention, and computes a partial output projection
out_partial = attn_heads @ Wo[hk block] in fp16.  The host sums the 4
partials per batch in fp32.

v3 (all-bf16, 4-slot interleaved pipeline):
- every matmul operand is bf16 (same PE rate as fp32r at these sizes, but
  FWL halves LDWEIGHTS and DMA traffic halves).
- per step s the PE stream is: scores(s-1) | projections(s) | den+attnV
  (s-1) | out-proj(s-2).  Each matmul group's cross-engine inputs were
  produced >= half a step earlier, so the PE never waits on the
  softmax/RoPE chain and HAM stays warm.
- all DRAM inputs are pre-laid out host-side as the exact SBUF tile
  images (16 KB contiguous per partition row), so a whole tensor loads
  as one large DMA that spreads across all 16 SDMA engines.
- q/k head blocks are transposed with DMA xbar transposes
  (dma_start_transpose, 2-byte dtype) on the ACT ring; bulk loads ride
  the SP ring so the latency-critical transposes never queue behind
  them.
- Wq/Wk columns are permuted on the host (evens then odds per head) so
  RoPE pairs become contiguous halves; q.k dot products are invariant.
- softmax: scores are bounded, so no max subtraction.  exp blocks are
  pre-added on DVE and one ones-vector matmul yields the denominator;
  the reciprocal is partition-broadcast on gpsimd and folded into the
  PSUM->SBUF move of the attention output.
"""

import sys

for _p in ("/opt/trn_rl_repo", "/opt/pypackages"):
    if _p not in sys.path:
        sys.path.insert(0, _p)

from contextlib import ExitStack

import numpy as np
import ml_dtypes

import concourse.bacc as bacc
import concourse.bass as bass
import concourse.mybir as mybir
import concourse.tile as tile
from concourse.bass_utils import run_bass_kernel_spmd

B, N, E = 2, 2048, 2048
H, G, WIN = 16, 4, 256
HK = H // G          # 4 kv heads
D = E // H           # 128
SCALE = D ** -0.5
NCORES = 8
P = 128
NT = N // P          # 16 row tiles
EC = E // P          # 16 contraction chunks
QTR = N // 4         # 512: x streamed in quarter-columns
HD = D // 2          # 64
W = G * P            # 512: (g, qi) moving width
F32 = mybir.dt.float32
BF16 = mybir.dt.bfloat16
FP16 = mybir.dt.float16
MASK_VAL = -1.0e30
NPBF16 = ml_dtypes.bfloat16

_compiled = {}


def _rope(nc, rtmp, dst, src_ap, c_ap, s_ap, nb):
    """RoPE on [128, nb*128] where each 128-block is [x0(64) | x1(64)]
    (host permuted the projection weights to evens-then-odds order)."""
    sv = src_ap.rearrange("p (b d) -> p b d", d=D)
    dv = dst[:].rearrange("p (b d) -> p b d", d=D)
    x0 = sv[:, :, 0:HD]
    x1 = sv[:, :, HD:D]
    d0 = dv[:, :, 0:HD]
    d1 = dv[:, :, HD:D]
    c3 = bass.AP(c_ap.tensor, c_ap.offset, [c_ap.ap[0], [0, nb], c_ap.ap[1]])
    s3 = bass.AP(s_ap.tensor, s_ap.offset, [s_ap.ap[0], [0, nb], s_ap.ap[1]])
    t0 = rtmp.tile([P, nb * HD], F32, tag="t0", name="t0")
    t1 = rtmp.tile([P, nb * HD], F32, tag="t1", name="t1")
    t0v = t0[:].rearrange("p (b d) -> p b d", d=HD)
    t1v = t1[:].rearrange("p (b d) -> p b d", d=HD)
    nc.vector.tensor_mul(t0v, x0, c3)
    nc.vector.tensor_mul(t1v, x1, s3)
    nc.vector.tensor_sub(d0, t0v, t1v)
    t2 = rtmp.tile([P, nb * HD], F32, tag="t2", name="t2")
    t3 = rtmp.tile([P, nb * HD], F32, tag="t3", name="t3")
    t2v = t2[:].rearrange("p (b d) -> p b d", d=HD)
    t3v = t3[:].rearrange("p (b d) -> p b d", d=HD)
    nc.vector.tensor_mul(t2v, x0, s3)
    nc.vector.tensor_mul(t3v, x1, c3)
    nc.vector.tensor_add(d1, t2v, t3v)


def _bcast_g(ap):
    """[128, 128] AP -> [128, G, 128] with a 0-step head dim."""
    return bass.AP(ap.tensor, ap.offset, [ap.ap[0], [0, G], ap.ap[1]])


def _build():
    nc = bacc.Bacc("TRN2", target_bir_lowering=False, debug=False)

    # all inputs are SBUF tile images: [128 partitions, free]
    xt_d = nc.dram_tensor("xti", [4, P, EC * QTR], BF16, kind="ExternalInput")
    wq_d = nc.dram_tensor("wqi", [P, EC * W], BF16, kind="ExternalInput")
    wkv_d = nc.dram_tensor("wkvi", [P, EC * 2 * D], BF16, kind="ExternalInput")
    wo_d = nc.dram_tensor("woi", [P, G * E], BF16, kind="ExternalInput")
    cos_d = nc.dram_tensor("cosi", [P, NT * HD], F32, kind="ExternalInput")
    sin_d = nc.dram_tensor("sini", [P, NT * HD], F32, kind="ExternalInput")
    maskt_d = nc.dram_tensor("maskt", [P, 2 * P], F32, kind="ExternalInput")
    onesr_d = nc.dram_tensor("onesr", [P, 1], BF16, kind="ExternalInput")
    out_d = nc.dram_tensor("out", [N, E], FP16, kind="ExternalOutput")

    with tile.TileContext(nc) as tc, ExitStack() as top:
        pers = top.enter_context(tc.tile_pool(name="pers", bufs=1))
        wq_sb = pers.tile([P, EC * W], BF16, tag="wq")        # [p, (e, 512)]
        wkv_sb = pers.tile([P, EC * 2 * D], BF16, tag="wkv")  # [p, (e, 256)]
        wo_sb = pers.tile([P, G * E], BF16, tag="wo")         # [p, (g, 2048)]
        cos_sb = pers.tile([P, NT * HD], F32, tag="cos")
        sin_sb = pers.tile([P, NT * HD], F32, tag="sin")
        maskt_sb = pers.tile([P, 2 * P], F32, tag="maskt")
        ones_sb = pers.tile([P, 1], BF16, tag="ones")
        kt_sb = pers.tile([P, N], BF16, tag="kt")             # [d, n]
        v_sb = pers.tile([P, N], BF16, tag="v")               # blk t: v[t*128+p, d]

        wq_v = wq_sb[:].rearrange("p (c m) -> p c m", m=W)
        wkv_v = wkv_sb[:].rearrange("p (c m) -> p c m", m=2 * D)
        wo_v = wo_sb[:].rearrange("p (g e) -> p g e", e=E)

        xq_pool = top.enter_context(tc.tile_pool(name="xqp", bufs=3))
        qt_pool = top.enter_context(tc.tile_pool(name="qtp", bufs=3))
        qrot_pool = top.enter_context(tc.tile_pool(name="qrot", bufs=3))
        krot_pool = top.enter_context(tc.tile_pool(name="krot", bufs=3))
        rtmp = top.enter_context(tc.tile_pool(name="rtmp", bufs=4))
        ex_pool = top.enter_context(tc.tile_pool(name="ex", bufs=6))
        es_pool = top.enter_context(tc.tile_pool(name="es", bufs=4))
        smm_pool = top.enter_context(tc.tile_pool(name="smm", bufs=2))
        stat_pool = top.enter_context(tc.tile_pool(name="stat", bufs=2))
        bc_pool = top.enter_context(tc.tile_pool(name="bcs", bufs=2))
        ao_pool = top.enter_context(tc.tile_pool(name="aosb", bufs=3))
        osb_pool = top.enter_context(tc.tile_pool(name="osb", bufs=2))

        ps_big = top.enter_context(tc.tile_pool(name="psb", bufs=5, space="PSUM"))
        ps_kv = top.enter_context(tc.tile_pool(name="psk", bufs=2, space="PSUM"))
        ps_den = top.enter_context(tc.tile_pool(name="psd", bufs=1, space="PSUM"))

        xqs = [None] * 4        # per-quarter x tiles [128, EC*512]
        qt_tiles = [None] * NT
        st_live = [None] * NT   # per-qt state handed between b_* slots

        def issue_xq(qtr, split):
            xq = xq_pool.tile([P, EC * QTR], BF16, tag="xq", name="xqt")
            xqs[qtr] = xq
            src = xt_d.ap()[qtr]
            if split:
                nc.sync.dma_start(xq[:, 0:2 * QTR], src[:, 0:2 * QTR])
                nc.sync.dma_start(xq[:, 2 * QTR:8 * QTR], src[:, 2 * QTR:8 * QTR])
                nc.sync.dma_start(xq[:, 8 * QTR:16 * QTR], src[:, 8 * QTR:16 * QTR])
            else:
                nc.sync.dma_start(xq[:, 0:8 * QTR], src[:, 0:8 * QTR])
                nc.sync.dma_start(xq[:, 8 * QTR:16 * QTR], src[:, 8 * QTR:16 * QTR])

        def a_part(T):
            qtr, tq = divmod(T, 4)
            if T == 0:
                # startup: smallest-first so tile 0's deps land early
                nc.sync.dma_start(wq_sb[:, 0:2 * W], wq_d.ap()[:, 0:2 * W])
                nc.sync.dma_start(wkv_sb[:, 0:2 * 2 * D],
                                  wkv_d.ap()[:, 0:2 * 2 * D])
                nc.sync.dma_start(maskt_sb[:], maskt_d.ap())
                nc.sync.dma_start(ones_sb[:], onesr_d.ap())
                issue_xq(0, split=True)
                nc.sync.dma_start(cos_sb[:], cos_d.ap())
                nc.sync.dma_start(sin_sb[:], sin_d.ap())
                nc.sync.dma_start(wq_sb[:, 2 * W:8 * W],
                                  wq_d.ap()[:, 2 * W:8 * W])
                nc.sync.dma_start(wkv_sb[:, 2 * 2 * D:16 * 2 * D],
                                  wkv_d.ap()[:, 2 * 2 * D:16 * 2 * D])
                nc.sync.dma_start(wq_sb[:, 8 * W:16 * W],
                                  wq_d.ap()[:, 8 * W:16 * W])
                nc.sync.dma_start(wo_sb[:, 0:2 * E], wo_d.ap()[:, 0:2 * E])
                nc.sync.dma_start(wo_sb[:, 2 * E:4 * E],
                                  wo_d.ap()[:, 2 * E:4 * E])
            if T in (1, 5, 9):
                issue_xq(T // 4 + 1, split=False)
            xq = xqs[qtr]

            # projections: q and k/v share each stationary x chunk
            q_ps = ps_big.tile([P, W], F32, tag="psb")
            kv_ps = ps_kv.tile([P, 2 * D], F32, tag="psk")
            for e in range(EC):
                lhsT = xq[:, e * QTR + tq * P: e * QTR + (tq + 1) * P]
                nc.tensor.matmul(
                    q_ps[:], lhsT, wq_v[:, e, :],
                    start=(e == 0), stop=(e == EC - 1))
                nc.tensor.matmul(
                    kv_ps[:], lhsT, wkv_v[:, e, :],
                    start=(e == 0), stop=(e == EC - 1))

            c_ap = cos_sb[:, T * HD:(T + 1) * HD]
            s_ap = sin_sb[:, T * HD:(T + 1) * HD]
            q_rot = qrot_pool.tile([P, W], BF16, tag="qrot")
            k_rot = krot_pool.tile([P, D], BF16, tag="krot")
            _rope(nc, rtmp, q_rot, q_ps[:], c_ap, s_ap, G)
            _rope(nc, rtmp, k_rot, kv_ps[:, 0:D], c_ap, s_ap, 1)
            nc.vector.tensor_copy(v_sb[:, T * P:(T + 1) * P], kv_ps[:, D:2 * D])

            # transposes on the DMA xbar; ACT ring (sync ring carries bulk)
            qt_t = qt_pool.tile([P, W], BF16, tag="qt")
            qt_tiles[T] = qt_t
            nc.scalar.dma_start_transpose(
                qt_t[:].rearrange("p (g n) -> p g n", g=G), q_rot[:])
            nc.scalar.dma_start_transpose(kt_sb[:, T * P:(T + 1) * P], k_rot[:])

        def b_scores(qt):
            """scores + exp for query tile qt (inputs ready a step ago)."""
            qt_t = qt_tiles[qt]
            nk = min(qt, 2) + 1
            kb0 = qt - (nk - 1)
            exps = [ex_pool.tile([P, W], BF16, tag="ex", name="exv")
                    for _ in range(nk)]
            for j in range(nk):
                kb = kb0 + j
                dabs = kb - qt          # -2, -1, or 0
                st_ps = ps_big.tile([P, W], F32, tag="psb")
                nc.tensor.matmul(
                    st_ps[:], kt_sb[:, kb * P:(kb + 1) * P], qt_t[:],
                    start=True, stop=True)
                if dabs == -1:
                    nc.scalar.activation(
                        exps[j][:], st_ps[:], mybir.ActivationFunctionType.Exp)
                else:
                    mblk = maskt_sb[:, 0:P] if dabs == -2 \
                        else maskt_sb[:, P:2 * P]
                    st_sb = smm_pool.tile([P, W], F32, tag="stsb")
                    nc.vector.tensor_add(
                        st_sb[:].rearrange("p (g q) -> p g q", g=G),
                        st_ps[:].rearrange("p (g q) -> p g q", g=G),
                        _bcast_g(mblk))
                    nc.scalar.activation(
                        exps[j][:], st_sb[:], mybir.ActivationFunctionType.Exp)
            # denominator pre-add on DVE
            if nk == 1:
                esum_ap = exps[0][:]
            elif nk == 2:
                es = es_pool.tile([P, W], BF16, tag="es", name="es")
                nc.vector.tensor_add(es[:], exps[0][:], exps[1][:])
                esum_ap = es[:]
            else:
                e01 = es_pool.tile([P, W], BF16, tag="es", name="e01")
                nc.vector.tensor_add(e01[:], exps[0][:], exps[1][:])
                es = es_pool.tile([P, W], BF16, tag="es", name="e012")
                nc.vector.tensor_add(es[:], e01[:], exps[2][:])
                esum_ap = es[:]
            st_live[qt] = (exps, esum_ap, nk, kb0)

        def b_tail(qt):
            """denominator matmul, reciprocal, attn@V, normalize."""
            exps, esum_ap, nk, kb0 = st_live[qt]
            den_ps = ps_den.tile([1, W], F32, tag="psd")
            nc.tensor.matmul(den_ps[:], ones_sb[:, 0:1], esum_ap,
                             start=True, stop=True)
            recip = stat_pool.tile([1, W], F32, tag="recip")
            nc.vector.reciprocal_approx_fast(recip[:], den_ps[:])
            bc_sb = bc_pool.tile([P, W], F32, tag="bcsb")
            nc.gpsimd.partition_broadcast(bc_sb[:], recip[:])

            ao_ps = ps_big.tile([P, W], F32, tag="psb")
            for j in range(nk):
                kb = kb0 + j
                nc.tensor.matmul(
                    ao_ps[:], v_sb[:, kb * P:(kb + 1) * P], exps[j][:],
                    start=(j == 0), stop=(j == nk - 1))
            ao_sb = ao_pool.tile([P, W], BF16, tag="aosb")
            nc.vector.tensor_mul(ao_sb[:], ao_ps[:], bc_sb[:])
            st_live[qt] = ao_sb

        def b_wo(qt):
            """output projection + store for query tile qt."""
            ao_sb = st_live[qt]
            out_sb = osb_pool.tile([P, E], FP16, tag="outsb")
            for eb in range(4):
                wo_ps = ps_big.tile([P, W], F32, tag="psb")
                for g in range(G):
                    nc.tensor.matmul(
                        wo_ps[:],
                        ao_sb[:, g * P:(g + 1) * P],
                        wo_v[:, g, eb * W:(eb + 1) * W],
                        start=(g == 0), stop=(g == G - 1))
                if eb % 2 == 0:
                    nc.scalar.copy(out_sb[:, eb * W:(eb + 1) * W], wo_ps[:])
                else:
                    nc.vector.tensor_copy(out_sb[:, eb * W:(eb + 1) * W],
                                          wo_ps[:])
            nc.scalar.dma_start(out_d.ap()[qt * P:(qt + 1) * P, :], out_sb[:])

        # 4-slot pipeline: scores lag A by 1 step, out-proj by 2
        for step in range(NT + 2):
            if 1 <= step <= NT:
                b_scores(step - 1)
            if step < NT:
                a_part(step)
            if 1 <= step <= NT:
                b_tail(step - 1)
            if step >= 2:
                b_wo(step - 2)

    nc.compile()
    return nc


_PERM = np.concatenate([np.arange(0, D, 2), np.arange(1, D, 2)])


def _sbuf_image(w, chunk):
    """[E, M] weight -> [128, (E/128, M)] SBUF image, contiguous rows."""
    rows = w.shape[0]
    return np.ascontiguousarray(
        w.reshape(rows // P, P, chunk).transpose(1, 0, 2).reshape(P, -1))


def _host_inputs(x, rope_cos, rope_sin, Wq, Wk, Wv, Wo):
    """Build the 8 per-core input maps (bf16, SBUF-image layouts)."""
    band = np.full((P, 3 * P), MASK_VAL, dtype=np.float32)
    r = np.arange(P)[:, None]
    c = np.arange(3 * P)[None, :]
    band[(c > r) & (c <= r + WIN)] = 0.0
    maskt = np.ascontiguousarray(np.concatenate(
        [band[:, 0:P].T, band[:, 2 * P:3 * P].T], axis=1))

    def permute_heads(w):
        nh = w.shape[1] // D
        w = w.reshape(E, nh, D)
        return w[:, :, _PERM].reshape(E, nh * D)

    in_maps = []
    for core in range(NCORES):
        b, hk = divmod(core, HK)
        xt = np.ascontiguousarray(x[b].T).astype(NPBF16)      # [E, N]
        # x image: [4 quarters, 128, (e, 512)]
        xti = np.ascontiguousarray(
            xt.reshape(EC, P, 4, QTR).transpose(2, 1, 0, 3).reshape(
                4, P, EC * QTR))
        wq = permute_heads(
            Wq[:, hk * G * D:(hk + 1) * G * D] * SCALE).astype(NPBF16)
        wk = permute_heads(Wk[:, hk * D:(hk + 1) * D])
        wkv = np.concatenate(
            [wk, Wv[:, hk * D:(hk + 1) * D]], axis=1).astype(NPBF16)
        wo = Wo[hk * G * D:(hk + 1) * G * D, :].astype(NPBF16)
        cos = rope_cos[b].astype(np.float32)                  # [N, 64]
        sin = rope_sin[b].astype(np.float32)
        in_maps.append({
            "xti": xti,
            "wqi": _sbuf_image(wq, G * D),
            "wkvi": _sbuf_image(wkv, 2 * D),
            "woi": _sbuf_image(wo, E),
            "cosi": _sbuf_image(cos, HD),
            "sini": _sbuf_image(sin, HD),
            "maskt": maskt,
            "onesr": np.ones((P, 1), dtype=NPBF16),
        })
    return in_maps


def _run(inputs, trace=False, **kw):
    if "nc" not in _compiled:
        _compiled["nc"] = _build()
    nc = _compiled["nc"]
    in_maps = _host_inputs(**inputs)
    res = run_bass_kernel_spmd(nc, in_maps, list(range(NCORES)), trace=trace, **kw)
    out = np.zeros((B, N, E), dtype=np.float32)
    for core in range(NCORES):
        b = core // HK
        out[b] += np.asarray(res.results[core]["out"]).astype(np.float32)
    return out, res


def kernel(**inputs):
    out, _ = _run(inputs, trace=False)
    return out
